# revision 18
# baseline (speedup 1.0000x reference)
"""Trainium2 Bass kernel for CausalSelfAttention (B=2, T=2048, C=1024, H=16).

Sharding: 8 cores = 2 batches x 4 head-groups (4 heads each).
Per core: QKV proj (k-major fp16, PE trails the x DMA stream) -> causal
attention (software-pipelined: S^T emitted one iteration ahead so
PE/ACT/DVE overlap across iterations) -> per-(slice,pair) fp16
AllGathers pipelined under subsequent attention -> output proj injected
into the attention instruction stream with split-k accumulation.
A leading AllReduce barrier absorbs inter-core startup skew.
"""

import sys

sys.path.insert(0, "/opt/trn_rl_repo")

import numpy as np

import concourse.bass as bass
import concourse.mybir as mybir
import concourse.tile as tile
from concourse import bacc
from concourse.bass_utils import run_bass_kernel_spmd

F32 = mybir.dt.float32
F16 = mybir.dt.float16

B, T, C, H = 2, 2048, 1024, 16
HD = C // H  # 64
N_CORES = 8
GROUPS = 4            # head groups (one per core within a batch)
HPG = H // GROUPS     # heads per group = 4
PAIRS = HPG // 2      # head pairs per core = 2
KEEP = 0.9
EXP_BIAS = -3.0       # exp(s - 3): cancels in normalization, avoids fp16 overflow

W = 512               # tq tile width
NJ = T // W           # 4 tq tiles
NT128 = T // 128      # 16 tk tiles of 128
KT = C // 128         # 8 contraction tiles


def build_kernel():
    nc = bacc.Bacc("TRN2", target_bir_lowering=False, debug=False,
                   num_devices=N_CORES)

    # ---- per-core DRAM I/O ----
    xT = nc.dram_tensor("xT", [C, T], F16, kind="ExternalInput")
    wqk = nc.dram_tensor("wqk", [C, 512], F16, kind="ExternalInput")
    wv = nc.dram_tensor("wv", [C, 256], F16, kind="ExternalInput")
    vbias = nc.dram_tensor("vbias", [128, 256], F32, kind="ExternalInput")
    bqk = nc.dram_tensor("bqk", [4, 128, 1], F32, kind="ExternalInput")
    wproj = nc.dram_tensor("wproj", [C, 256], F16, kind="ExternalInput")
    bpr = nc.dram_tensor("bpr", [2, 128, 1], F32, kind="ExternalInput")
    maskT = nc.dram_tensor("maskT", [T, HPG, T], F16, kind="ExternalInput")
    rmaskT = nc.dram_tensor("rmaskT", [256, T], F16, kind="ExternalInput")
    triu_in = nc.dram_tensor("triu", [128, 128], F16, kind="ExternalInput")
    out = nc.dram_tensor("out", [256, T], F32, kind="ExternalOutput")

    # startup-skew barrier buffers + per-(slice,pair) AllGather staging
    bar_in = nc.dram_tensor("bar_in", [128, 1], F16)
    bar_out = nc.dram_tensor("bar_out", [128, 1], F16)
    ag_in = [[nc.dram_tensor(f"ag_in{j}_{p}", [128, W], F16)
              for p in range(PAIRS)] for j in range(NJ)]
    ag_out = [[nc.dram_tensor(f"ag_out{j}_{p}", [512, W], F16)
               for p in range(PAIRS)] for j in range(NJ)]

    with tile.TileContext(nc) as tc:
        _build_body(nc, tc, xT, wqk, wv, vbias, bqk, wproj, bpr, maskT,
                    rmaskT, triu_in, out, bar_in, bar_out, ag_in, ag_out)
    nc.compile()
    return nc


def _build_body(nc, tc, xT, wqk, wv, vbias, bqk, wproj, bpr, maskT,
                rmaskT, triu_in, out, bar_in, bar_out, ag_in, ag_out):
    from contextlib import ExitStack
    ctx = ExitStack()

    # ---- PSUM (8 banks, managed manually) ----
    psS = [ctx.enter_context(nc.psum_tensor(f"psS{b}", [128, 1024], F32))
           for b in range(2)]                                    # banks 0-3
    ps_av = ctx.enter_context(nc.psum_tensor([128, 512], F32))   # bank 4
    ps_cs = ctx.enter_context(nc.psum_tensor([128, 512], F32))   # bank 5
    ps_pr = [ctx.enter_context(nc.psum_tensor(f"psPr{b}", [128, 512], F32))
             for b in range(2)]                                  # banks 6-7

    # 8 phase-1 regions of [128, 512] (one per bank) for k-major accumulation
    p1reg = [psS[0][:, 0:512], psS[0][:, 512:1024],
             psS[1][:, 0:512], psS[1][:, 512:1024],
             ps_av[:], ps_cs[:], ps_pr[0][:], ps_pr[1][:]]

    # ---- persistent SBUF ----
    big = ctx.enter_context(tc.tile_pool(name="big", bufs=1))
    qT_sb = [big.tile([128, T], F16, name=f"qT{p}") for p in range(PAIRS)]
    kT_sb = [big.tile([128, T], F16, name=f"kT{p}") for p in range(PAIRS)]
    v_sb = big.tile([128, NT128 * 256], F16, name="v")
    vbias_sb = big.tile([128, 256], F32, name="vbias")
    bqk_sb = [big.tile([128, 1], F32, name=f"bqk{m}") for m in range(4)]
    bpr_sb = [big.tile([128, 1], F32, name=f"bpr{m}") for m in range(2)]
    rmask_sb = [big.tile([128, T], F16, name=f"rm{m}") for m in range(2)]
    wproj_sb = [big.tile([128, 256], F16, name=f"wp{k}") for k in range(KT)]
    triu_sb = big.tile([128, 128], F16, name="triu")
    ones_cs = big.tile([128, 1], F16, name="ones_cs")
    ones_b = big.tile([128, 1], F16, name="ones_b")
    expb_sb = big.tile([128, 1], F32, name="expb")

    # ---- rotating SBUF pools ----
    mpool = ctx.enter_context(tc.tile_pool(name="mask", bufs=6))
    apool = ctx.enter_context(tc.tile_pool(name="araw", bufs=3))
    dpool = ctx.enter_context(tc.tile_pool(name="adrop", bufs=3))
    spool = ctx.enter_context(tc.tile_pool(name="small", bufs=2))

    # phase-1-only tensors live in their own pool; space reused by yall later
    xpool = tc.alloc_tile_pool(name="xpool", bufs=1)
    xT_sb = [xpool.tile([128, T], F16, name=f"xT{k}") for k in range(KT)]
    wqk_sb = [xpool.tile([128, 512], F16, name=f"wqk{k}") for k in range(KT)]
    wv_sb = [xpool.tile([128, 256], F16, name=f"wv{k}") for k in range(KT)]

    nc.vector.memset(ones_cs[:], 1.0)

    # ---- input DMA: x/wqk/wv first (phase 1 consumes them k-by-k) ----
    for k in range(KT):
        nc.sync.dma_start(xT_sb[k][:], xT[k * 128:(k + 1) * 128, :])
        nc.sync.dma_start(wv_sb[k][:], wv[k * 128:(k + 1) * 128, :])
        nc.sync.dma_start(wqk_sb[k][:], wqk[k * 128:(k + 1) * 128, :])
    nc.sync.dma_start(vbias_sb[:], vbias[:, :])
    for m in range(4):
        nc.sync.dma_start(bqk_sb[m][:], bqk.ap()[m])
    nc.sync.dma_start(triu_sb[:], triu_in[:, :])
    for m in range(2):
        nc.sync.dma_start(bpr_sb[m][:], bpr.ap()[m])
        nc.sync.dma_start(rmask_sb[m][:], rmaskT[m * 128:(m + 1) * 128, :])
    for k in range(KT):
        nc.sync.dma_start(wproj_sb[k][:], wproj[k * 128:(k + 1) * 128, :])
    nc.vector.memset(expb_sb[:], EXP_BIAS)

    # ================= Phase 1: QKV projection (k-major) =================
    # V first: PE trails the x DMAs. One accumulation group per PSUM bank
    # (start=True zeroes the whole 2KB zero region), so two passes of 8.
    for vpass in range(2):
        for k in range(KT):
            for nn in range(8):
                n = 8 * vpass + nn
                nc.tensor.matmul(
                    p1reg[nn][:, 0:256],
                    xT_sb[k][:, n * 128:(n + 1) * 128],
                    wv_sb[k][:, 0:256],
                    start=(k == 0), stop=(k == KT - 1))
        for nn in range(8):
            n = 8 * vpass + nn
            nc.vector.tensor_tensor(
                v_sb[:, n * 256:(n + 1) * 256], p1reg[nn][:, 0:256],
                vbias_sb[:], mybir.AluOpType.add)

    # Q then K: two k-major passes of 8 regions of [128, 512] each.
    # m: 0=q-pair0, 1=q-pair1, 2=k-pair0, 3=k-pair1; q-scale folded on host.
    for half in range(2):               # 0: q (m=0,1), 1: k (m=2,3)
        for k in range(KT):
            for mm in range(2):
                m = 2 * half + mm
                for n in range(NJ):
                    nc.tensor.matmul(
                        p1reg[4 * mm + n],
                        wqk_sb[k][:, m * 128:(m + 1) * 128],
                        xT_sb[k][:, n * 512:(n + 1) * 512],
                        start=(k == 0), stop=(k == KT - 1))
        for mm in range(2):
            m = 2 * half + mm
            dest = qT_sb[mm] if half == 0 else kT_sb[mm]
            for n in range(NJ):
                nc.scalar.activation(
                    dest[:, n * 512:(n + 1) * 512], p1reg[4 * mm + n],
                    mybir.ActivationFunctionType.Identity,
                    bias=bqk_sb[m][:, 0:1], scale=1.0)

    # xT no longer needed: release its pool so yall reuses the space
    xpool.release()
    ypool = ctx.enter_context(tc.tile_pool(name="ypool", bufs=1))

    # ============ Phase 2+3+4: attention w/ pipelined AG + proj ==========
    yall = {}  # (j, p) -> sbuf tile [128, 4*W] of gathered pair-p features

    def emit_ag(j, p):
        nc.gpsimd.collective_compute(
            "AllGather", mybir.AluOpType.bypass,
            replica_groups=[[0, 1, 2, 3], [4, 5, 6, 7]],
            ins=[ag_in[j][p].ap()], outs=[ag_out[j][p].ap()])
        t = ypool.tile([128, 4 * W], F16, tag=f"yall{(j % 2) * 2 + p}")
        for r in range(4):
            nc.sync.dma_start(t[:, r * W:(r + 1) * W],
                              ag_out[j][p][r * 128:(r + 1) * 128, :])
        yall[(j, p)] = t

    def emit_proj_part(j, m, part):
        # co-tile m of out slice j, contraction k-tiles of pair `part`
        ps = ps_pr[m]
        for r in range(4):
            kk = 4 * part + r
            nc.tensor.matmul(
                ps[:],
                wproj_sb[kk][:, m * 128:(m + 1) * 128],
                yall[(j, part)][:, r * W:(r + 1) * W],
                start=(kk == 0), stop=(kk == KT - 1))
        if part == 1:
            t_m = spool.tile([128, W], F32, tag="tproj")
            nc.scalar.activation(t_m[:], ps[:],
                                 mybir.ActivationFunctionType.Identity,
                                 bias=bpr_sb[m][:, 0:1], scale=1.0 / KEEP)
            o_m = spool.tile([128, W], F32, tag="oproj")
            nc.vector.tensor_tensor(
                o_m[:], t_m[:],
                rmask_sb[m][:, j * 512:(j + 1) * 512],
                mybir.AluOpType.mult)
            nc.sync.dma_start(out[m * 128:(m + 1) * 128,
                                  j * 512:(j + 1) * 512], o_m[:])

    # proj work queue: (j, m, part) pieces consumed at scheduled points.
    # pieces(j) are consumed at (j+2, p0) [one slice of cushion for the AG
    # chain]; j=2 pieces at (3, p1) early points, j=3 pair-0 parts at
    # (3, p1) late points, j=3 pair-1 parts in the tail.
    proj_q = []
    SCHED = {(2, 1): (3, 5, 7, 9), (3, 0): (3, 5, 7, 9),
             (3, 1): (3, 5, 7, 9, 11, 14)}

    def inject_proj(n_pieces):
        for _ in range(n_pieces):
            if proj_q:
                emit_proj_part(*proj_q.pop(0))

    def emit_st(p, j, i):
        r = max(0, i - 4 * j)
        w_mm = max(512 - 128 * r, 256)
        off_mm = 512 - w_mm
        ps_s = psS[i % 2]
        for h in range(2):
            nc.tensor.matmul(
                ps_s[:, h * 512 + off_mm:h * 512 + 512],
                kT_sb[p][h * 64:(h + 1) * 64, i * 128:(i + 1) * 128],
                qT_sb[p][h * 64:(h + 1) * 64,
                         j * 512 + off_mm:(j + 1) * 512],
                start=True, stop=True)

    for j in range(NJ):
        for p in range(PAIRS):
            n_i = 4 * j + 4  # tk tiles needed (block-causal)
            pts = SCHED.get((j, p), ())
            emit_st(p, j, 0)
            for i in range(n_i):
                r = max(0, i - 4 * j)
                off = 128 * r
                w = 512 - off
                ps_s = psS[i % 2]
                # --- dropout+causal mask tile (both heads, one DMA) ---
                m_tile = mpool.tile([128, 1024], F16, tag="mask")
                mt_v = m_tile[:].rearrange("t (h q) -> t h q", h=2)
                nc.sync.dma_start(
                    mt_v[:, :, off:512],
                    maskT[i * 128:(i + 1) * 128, 2 * p:2 * p + 2,
                          j * 512 + off:(j + 1) * 512])
                # --- next iteration's S^T (software pipelining: keeps the
                # PE a full iteration ahead of the exp/mask/AV chain) ---
                if i + 1 < n_i:
                    emit_st(p, j, i + 1)
                if i in pts:
                    inject_proj(1)
                # --- exp ---
                a_raw = apool.tile([128, 1024], F16, tag="araw")
                if off == 0:
                    nc.scalar.activation(
                        a_raw[:], ps_s[:],
                        mybir.ActivationFunctionType.Exp,
                        bias=expb_sb[:, 0:1], scale=1.0)
                else:
                    for h in range(2):
                        nc.scalar.activation(
                            a_raw[:, h * 512 + off:h * 512 + 512],
                            ps_s[:, h * 512 + off:h * 512 + 512],
                            mybir.ActivationFunctionType.Exp,
                            bias=expb_sb[:, 0:1], scale=1.0)
                # --- causal triangle on diagonal blocks ---
                if i >= 4 * j:
                    for h in range(2):
                        sl = a_raw[:, h * 512 + off:h * 512 + off + 128]
                        nc.vector.tensor_tensor(sl, sl, triu_sb[:],
                                                mybir.AluOpType.mult)
                # --- dropout mask multiply ---
                a_drop = dpool.tile([128, 1024], F16, tag="adrop")
                ad_v = a_drop[:].rearrange("t (h q) -> t h q", h=2)[:, :, off:512]
                ar_v = a_raw[:].rearrange("t (h q) -> t h q", h=2)[:, :, off:512]
                nc.vector.tensor_tensor(ad_v, ar_v, mt_v[:, :, off:512],
                                        mybir.AluOpType.mult)
                # --- colsum (denominator): head A -> row 0, head B -> row 32
                for h in range(2):
                    nc.tensor.matmul(
                        ps_cs[32 * h:32 * h + 1, off:512],
                        ones_cs[:],
                        a_raw[:, h * 512 + off:h * 512 + 512],
                        start=(i == 0),
                        stop=(i == n_i - 1),
                        skip_group_check=True)
                # --- AV (heads stacked in one bank) ---
                for h in range(2):
                    nc.tensor.matmul(
                        ps_av[64 * h:64 * h + 64, off:512],
                        v_sb[:, i * 256 + (2 * p + h) * 64:
                             i * 256 + (2 * p + h) * 64 + 64],
                        a_drop[:, h * 512 + off:h * 512 + 512],
                        start=(i == 0),
                        stop=(i == n_i - 1),
                        skip_group_check=True)
            # --- normalize: y = (a_drop @ v/KEEP) / den ---
            # fast fp32 reciprocal (DVE custom op) + gpsimd row-broadcast;
            # keeps the PE stream free of normalization work.
            recipA = spool.tile([1, 512], F32, tag="recipA")
            recipB = spool.tile([1, 512], F32, tag="recipB")
            denB = spool.tile([1, 512], F32, tag="denB")
            # the custom-DVE reciprocal mishandles partition-offset PSUM APs;
            # stage head B's denominator at partition 0 first
            nc.vector.tensor_copy(denB[:], ps_cs[32:33, :])
            nc.vector.reciprocal_approx_fast(recipA[:], ps_cs[0:1, :])
            nc.vector.reciprocal_approx_fast(recipB[:], denB[:])
            bcA = spool.tile([128, W], F32, tag="bcastA")
            bcB = spool.tile([128, W], F32, tag="bcastB")
            nc.gpsimd.partition_broadcast(bcA[:], recipA[:], channels=128)
            nc.gpsimd.partition_broadcast(bcB[:], recipB[:], channels=128)
            ynorm = spool.tile([128, W], F16, tag="ynorm")
            nc.vector.tensor_tensor(ynorm[0:64, :], ps_av[0:64, :],
                                    bcA[0:64, :], mybir.AluOpType.mult)
            nc.vector.tensor_tensor(ynorm[64:128, :], ps_av[64:128, :],
                                    bcB[64:128, :], mybir.AluOpType.mult)
            nc.sync.dma_start(ag_in[j][p][:, :], ynorm[:])
            emit_ag(j, p)
            if j == NJ - 1 and p == 0:
                # last slice: pair-0 proj parts go into pair-1's attention
                proj_q.extend([(j, 0, 0), (j, 1, 0)])
        if j < NJ - 1:
            proj_q.extend([(j, 0, 0), (j, 1, 0), (j, 0, 1), (j, 1, 1)])

    # tail: flush any unconsumed pieces, then the last slice's pair-1 halves
    while proj_q:
        emit_proj_part(*proj_q.pop(0))
    emit_proj_part(NJ - 1, 0, 1)
    emit_proj_part(NJ - 1, 1, 1)

    ctx.close()


def prep_inputs(x, Wqkv, bqkv, Wproj, bproj, attn_drop_mask, resid_drop_mask):
    """Shard + lay out the full inputs for the 8 cores."""
    x = np.asarray(x, np.float32)
    Wqkv = np.asarray(Wqkv, np.float32)
    bqkv = np.asarray(bqkv, np.float32)
    Wproj = np.asarray(Wproj, np.float32)
    bproj = np.asarray(bproj, np.float32)
    attn_drop_mask = np.asarray(attn_drop_mask, bool)
    resid_drop_mask = np.asarray(resid_drop_mask, bool)

    tril = np.tril(np.ones((T, T), dtype=bool))
    triu128 = np.triu(np.ones((128, 128), np.float16))
    sq = np.float32(1.0 / np.sqrt(HD))
    in_maps = []
    for core in range(N_CORES):
        b, g = divmod(core, GROUPS)
        cs = slice(g * 256, (g + 1) * 256)  # this group's feature rows
        # q-scale 1/sqrt(hd) folded into Wq/bq on host
        wqk_c = np.concatenate([Wqkv[:, cs] * sq, Wqkv[:, 1024:2048][:, cs]],
                               axis=1)
        # fold attention-dropout 1/KEEP into the V projection
        wv_c = np.ascontiguousarray(Wqkv[:, 2048:3072][:, cs]) / np.float32(KEEP)
        bq = (bqkv[0:1024][cs] * sq).astype(np.float32)
        bk = bqkv[1024:2048][cs]
        bv = bqkv[2048:3072][cs] / np.float32(KEEP)
        bqk_c = np.stack([bq[0:128], bq[128:256], bk[0:128], bk[128:256]])
        bqk_c = bqk_c.reshape(4, 128, 1)
        vbias_c = np.broadcast_to(bv, (128, 256)).copy()
        # combined causal & dropout mask, [tk, h, tq] layout, fp16
        m = attn_drop_mask[b, g * HPG:(g + 1) * HPG] & tril
        maskT_c = np.ascontiguousarray(
            m.transpose(2, 0, 1)).astype(np.float16)
        rmaskT_c = np.ascontiguousarray(
            resid_drop_mask[b, :, cs].T).astype(np.float16)
        bpr_c = (bproj[cs] / KEEP).astype(np.float32).reshape(2, 128, 1)
        # wproj rows permuted pair-major to match per-pair AllGather output:
        # k-tile kk = 4*p + r holds rows [256r+128p : 256r+128(p+1))
        wp_rows = []
        for p in range(2):
            for r in range(4):
                wp_rows.append(Wproj[256 * r + 128 * p:
                                     256 * r + 128 * (p + 1), cs])
        wproj_c = np.concatenate(wp_rows, axis=0)
        in_maps.append(dict(
            xT=np.ascontiguousarray(x[b].T).astype(np.float16),
            wqk=np.ascontiguousarray(wqk_c).astype(np.float16),
            wv=wv_c.astype(np.float16),
            vbias=vbias_c.astype(np.float32),
            bqk=bqk_c.astype(np.float32),
            wproj=np.ascontiguousarray(wproj_c).astype(np.float16),
            bpr=bpr_c,
            maskT=maskT_c,
            rmaskT=rmaskT_c,
            triu=triu128,
        ))
    return in_maps


_NC_CACHE = {}


def _get_nc():
    if "nc" not in _NC_CACHE:
        _NC_CACHE["nc"] = build_kernel()
    return _NC_CACHE["nc"]


def kernel(trace=False, **inputs):
    nc = _get_nc()
    in_maps = prep_inputs(**inputs)
    res = run_bass_kernel_spmd(nc, in_maps, core_ids=list(range(N_CORES)),
                               trace=trace)
    y = np.empty((B, T, C), np.float32)
    for core in range(N_CORES):
        b, g = divmod(core, GROUPS)
        y[b, :, g * 256:(g + 1) * 256] = res.results[core]["out"].T
    kernel.last_result = res
    return y
